# revision 12
# baseline (speedup 1.0000x reference)
"""CrossTeacherAttention Trainium2 kernel (fp8 DoubleRow rewrite).

Math per batch element b (x as [C=256, N=1024], N=H*W):
  Q  = Wq Xs + bq                       [C,N]
  G  = Wk^T Q                           [C,N]   (so S_t = Xt^T G: the three
                                                 K-projections fold into one)
  S_t[m,n] = sum_c Xt[c,m] G[c,n]
  E_t = exp(S_t/16 - 1.5)               (the -1.5 keeps E in fp8e4 range and
                                         cancels in the softmax normalization;
                                         bk shifts S per-n only -> provably no
                                         effect on the output, dropped)
  V_t^T = Xt^T Wv^T                     [N,C]  (bv deferred to the residual)
  O_t[n,c] (n-partition-major) = sum_m E_t[m,n] V_t^T[m,c], with a 257th
    moving column of constant 3.0 producing Z3_t[n] = 3*sum_m E_t[m,n] in the
    same PSUM accumulation.
  out^T = (Xs^T + bv) + sum_t O_t[:, :256] / Z3_t    (teacher weights are
    exactly 1/3: attn.mean(-1) of a softmax is 1/N, softmax over t of equal
    values is 1/3 -- folded into the 3.0 ones-column)

All five matmul families run as fp8e4 MatmulPerfMode.DoubleRow (K=256 per
instruction at 0.5 cycles/row).  Operands are packed [128, 2, F] with logical
contraction index k = p + 128*j.  exp runs on ACT as [128,1024] instructions
reading a 2-bank PSUM tile.  Normalization is per-partition (n on partitions):
DVE reciprocal of the fused Z3 column + scalar_tensor_tensor (O*recip + acc),
split across DVE and GPSIMD.  The residual Xs^T + bv is computed on host,
shipped as bf16 directly into the accumulator tile; output returns bf16
[128, 8, 256] (n-major) and the host unpacks/transposes/upcasts.

Sharding: data-parallel over batch, B=8 -> one batch element per core.
"""

import sys

sys.path.insert(0, "/opt/trn_rl_repo")

import ml_dtypes
import numpy as np

import concourse.bass as bass
import concourse.tile as tile
from concourse import mybir
from concourse.bass_utils import run_bass_kernel_spmd

B, C, H, W = 8, 256, 32, 32
N = H * W  # 1024
T = 3
P = 128
F32 = mybir.dt.float32
BF16 = mybir.dt.bfloat16
F8 = mybir.dt.float8e4
F8E5 = mybir.dt.float8e5
NP_F8 = ml_dtypes.float8_e4m3
NP_BF16 = ml_dtypes.bfloat16
SCALE = C ** -0.5  # 1/16
EXP_BIAS = -1.5
DR = mybir.MatmulPerfMode.DoubleRow


def build_nc():
    nc = bass.Bass()
    wpk_d = nc.dram_tensor("wpk", [P, 2, 2 * C], F8, kind="ExternalInput")
    xsp_d = nc.dram_tensor("xsp", [P, 2, N], F8, kind="ExternalInput")
    xt_d = nc.dram_tensor("xt", [T, P, 2, N], F8, kind="ExternalInput")
    gb_d = nc.dram_tensor("gb", [P, 2], F32, kind="ExternalInput")
    acc_d = nc.dram_tensor("accin", [P, 8, C], BF16, kind="ExternalInput")
    out_d = nc.dram_tensor("out", [P, 8, C], BF16, kind="ExternalOutput")

    with tile.TileContext(nc) as tc:
        with (
            tc.tile_pool(name="consts", bufs=1) as consts,
            tc.tile_pool(name="epool", bufs=12) as epool,
            tc.tile_pool(name="vpool", bufs=12) as vpool,
            tc.tile_pool(name="rpool", bufs=4) as rpool,
            tc.tile_pool(name="ps", bufs=2, space="PSUM") as ps,
            tc.tile_pool(name="pv", bufs=2, space="PSUM") as pv,
            tc.tile_pool(name="po", bufs=2, space="PSUM") as po,
        ):
            # ---- input loads (spread across engine DGE queues) ----
            wpk = consts.tile([P, 2, 2 * C], F8, tag="wpk", name="wpk")
            nc.sync.dma_start(out=wpk, in_=wpk_d[:, :, :])
            xsp = consts.tile([P, 2, N], F8, tag="xsp", name="xsp")
            nc.scalar.dma_start(out=xsp, in_=xsp_d[:, :, :])
            xt0 = consts.tile([P, 2, N], F8, tag="xt0", name="xt0")
            nc.gpsimd.dma_start(out=xt0, in_=xt_d[0])
            gb_sb = consts.tile([P, 2], F32, tag="gb", name="gb_sb")
            nc.sync.dma_start(out=gb_sb, in_=gb_d[:, :])
            acc = consts.tile([P, 8, C], BF16, tag="acc", name="acc")
            nc.scalar.dma_start(out=acc, in_=acc_d[:, :, :])
            xt1 = consts.tile([P, 2, N], F8, tag="xt1", name="xt1")
            nc.gpsimd.dma_start(out=xt1, in_=xt_d[1])
            xt2 = consts.tile([P, 2, N], F8, tag="xt2", name="xt2")
            nc.gpsimd.dma_start(out=xt2, in_=xt_d[2])
            xts = [xt0, xt1, xt2]

            ebias = consts.tile([P, 1], F32, tag="ebias", name="ebias")
            nc.vector.memset(ebias, EXP_BIAS)

            wm = wpk[:, :, 0:C]
            wv = wpk[:, :, C:2 * C]

            # ---- G = M Xs + gb (M = Wk^T Wq, gb = Wk^T bq, both host-side)
            # nh0 casts on DVE, nh1 on ACT so the two halves pipeline in
            # parallel during startup ----
            gf = consts.tile([P, 2, N], F8, tag="gf", name="gf")
            for nh in range(2):
                for co in range(2):
                    gp = ps.tile([P, 1024], F32, tag="s", name="gp")
                    nc.tensor.matmul(
                        gp[:, :512],
                        wm[:, :, co * P:(co + 1) * P],
                        xsp[:, :, nh * 512:(nh + 1) * 512],
                        start=True, stop=True, perf_mode=DR,
                    )
                    nc.vector.tensor_scalar_add(
                        gf[:, co, nh * 512:(nh + 1) * 512],
                        gp[:, :512],
                        gb_sb[:, co:co + 1],
                    )

            e_tiles = [[None] * 4 for _ in range(T)]
            v_tiles = [[None] * 4 for _ in range(T)]

            def emit_v(t):
                # V_t^T = Xt^T Wv^T -> v_aug [m-part, 2, 257] fp8 with a
                # 257th column of 3.0 (fused 3*Z row-sum weights)
                for r in range(4):
                    va = vpool.tile([P, 2, C + 1], F8, tag="v",
                                    name=f"v{t}{r}")
                    v_tiles[t][r] = va
                    nc.gpsimd.memset(va[:, :, C:C + 1], 3.0)
                    for j in range(2):
                        mi = 2 * r + j
                        vp = pv.tile([P, 512], F32, tag="v", name="vp")
                        nc.tensor.matmul(
                            vp[:, :C],
                            xts[t][:, :, mi * P:(mi + 1) * P],
                            wv,
                            start=True, stop=True, perf_mode=DR,
                        )
                        nc.vector.tensor_copy(va[:, j, :C], vp[:, :C])

            def emit_s(t):
                # S_t = Xt^T G -> exp -> packed e tiles [m-part, 2, 1024] fp8
                for mi in range(8):
                    r, j = divmod(mi, 2)
                    if j == 0:
                        e_tiles[t][r] = epool.tile([P, 2, N], F8E5, tag="e",
                                                   name=f"e{t}{r}")
                    sp = ps.tile([P, 1024], F32, tag="s", name="sp")
                    for nh in range(2):
                        nc.tensor.matmul(
                            sp[:, nh * 512:(nh + 1) * 512],
                            xts[t][:, :, mi * P:(mi + 1) * P],
                            gf[:, :, nh * 512:(nh + 1) * 512],
                            start=True, stop=True, perf_mode=DR,
                        )
                    if t == 0 and mi == 0:
                        # split halves: the first exp only waits on gf nh0
                        for nh in range(2):
                            nc.scalar.activation(
                                e_tiles[t][r][:, j, nh * 512:(nh + 1) * 512],
                                sp[:, nh * 512:(nh + 1) * 512],
                                func=mybir.ActivationFunctionType.Exp,
                                bias=ebias,
                                scale=SCALE,
                            )
                    else:
                        nc.scalar.activation(
                            e_tiles[t][r][:, j, :],
                            sp,
                            func=mybir.ActivationFunctionType.Exp,
                            bias=ebias,
                            scale=SCALE,
                        )

            def emit_o(t):
                # O_t [n-part, 257] += e_chunk^T @ v_aug over 4 m-pairs;
                # col 256 = 3*Z.  Normalize per-partition and accumulate.
                for w in range(2):
                    ops = []
                    for i in range(4):
                        opool, otag = (po, "o") if i % 2 == 0 else (pv, "v")
                        ops.append(opool.tile([P, 512], F32, tag=otag,
                                              name="op"))
                    for r in range(4):
                        for i in range(4):
                            nc.tensor.matmul(
                                ops[i][:, :C + 1],
                                e_tiles[t][r][:, :, (4 * w + i) * P:
                                              (4 * w + i + 1) * P],
                                v_tiles[t][r],
                                start=(r == 0), stop=(r == 3), perf_mode=DR,
                            )
                    for i in range(4):
                        nk = 4 * w + i
                        rp = rpool.tile([P, 1], F32, tag="r", name="rp")
                        nc.vector.reciprocal(rp, ops[i][:, C:C + 1])
                        nc.vector.scalar_tensor_tensor(
                            out=acc[:, nk, :],
                            in0=ops[i][:, :C],
                            scalar=rp,
                            in1=acc[:, nk, :],
                            op0=mybir.AluOpType.mult,
                            op1=mybir.AluOpType.add,
                        )

            # Emission order keeps PE streaming while ACT works through exps:
            # S(t0) V(t0) S(t1) V(t1) O(t0) S(t2) V(t2) O(t1) O(t2)
            emit_s(0)
            emit_v(0)
            emit_s(1)
            emit_v(1)
            emit_o(0)
            emit_s(2)
            emit_v(2)
            emit_o(1)
            emit_o(2)

            nc.gpsimd.dma_start(out=out_d[:, 0:4, :], in_=acc[:, 0:4, :])
            nc.gpsimd.dma_start(out=out_d[:, 4:6, :], in_=acc[:, 4:6, :])
            nc.gpsimd.dma_start(out=out_d[:, 6:8, :], in_=acc[:, 6:8, :])

    _split_multi_waits(nc)
    if not nc.is_finalized():
        nc.finalize()
    return nc


def _split_multi_waits(nc):
    """walrus can encode at most one sync-wait per instruction. Hoist every
    wait of a multi-wait instruction onto single-wait nops on the same
    engine, placed immediately before it in program order."""
    fixes = []
    for fn in nc.m.functions:
        for blk in fn.blocks:
            for inst in blk.instructions:
                si = getattr(inst, "sync_info", None)
                if (si is not None and si.on_wait and len(si.on_wait) > 1
                        and getattr(inst, "engine", None) is not None):
                    fixes.append((blk, inst))
    for blk, inst in fixes:
        si = inst.sync_info
        waits = list(si.on_wait)
        nops = []
        for w in waits:
            nop = nc.engines[inst.engine].nop(nofuse=True).ins
            nop.sync_info = mybir.SyncInfo(on_wait=[w], on_update=[])
            nops.append(nop)
        inst.sync_info = mybir.SyncInfo(on_wait=[], on_update=list(si.on_update))
        nop_names = {n.name for n in nops}
        for fn2 in nc.m.functions:
            for blk2 in fn2.blocks:
                blk2.instructions = [
                    i for i in blk2.instructions if i.name not in nop_names
                ]
        pos = next(i for i, x in enumerate(blk.instructions)
                   if x.name == inst.name)
        blk.instructions = (blk.instructions[:pos] + nops
                            + blk.instructions[pos:])


_NC = None


def _get_nc():
    global _NC
    if _NC is None:
        _NC = build_nc()
    return _NC


def _pack2(a):
    """[256, X] row-major -> [128, 2, X] with row c at [c % 128, c // 128]."""
    return np.ascontiguousarray(a.reshape(2, P, -1).transpose(1, 0, 2))


def make_in_maps(student_feat, t_feat0, t_feat1, t_feat2,
                 Wq, bq, Wk, bk, Wv, bv):
    xs = np.asarray(student_feat, np.float32).reshape(B, C, N)
    xt = np.ascontiguousarray(
        np.stack([t_feat0, t_feat1, t_feat2], axis=1), np.float32
    ).reshape(B, T, C, N)
    wq32 = np.asarray(Wq, np.float32)
    wk32 = np.asarray(Wk, np.float32)
    m = wk32.T @ wq32  # G = M Xs + gb folds the Q projection away
    gb = wk32.T @ np.asarray(bq, np.float32)
    wpk = np.concatenate(
        [
            _pack2(m.T.astype(NP_F8)),
            _pack2(np.asarray(Wv, np.float32).T.astype(NP_F8)),
        ],
        axis=2,
    )
    gbp = np.ascontiguousarray(gb.reshape(2, P).T)
    bv32 = np.asarray(bv, np.float32)
    maps = []
    for b in range(B):
        xsp = _pack2(xs[b].astype(NP_F8))
        xtp = np.stack([_pack2(xt[b, t].astype(NP_F8)) for t in range(T)])
        accin = np.ascontiguousarray(
            (xs[b].T + bv32[None, :]).reshape(8, P, C).transpose(1, 0, 2)
        ).astype(NP_BF16)
        maps.append({"wpk": wpk, "xsp": xsp, "xt": xtp, "gb": gbp,
                     "accin": accin})
    return maps


def run(in_maps, trace=False):
    nc = _get_nc()
    return run_bass_kernel_spmd(nc, in_maps, core_ids=list(range(B)),
                                trace=trace)


def unpack_out(raw):
    """[128, 8, 256] bf16 n-major -> [C, H, W] f32."""
    o = np.asarray(raw).astype(np.float32).transpose(1, 0, 2).reshape(N, C)
    return np.ascontiguousarray(o.T).reshape(C, H, W)


def kernel(student_feat, t_feat0, t_feat1, t_feat2,
           Wq, bq, Wk, bk, Wv, bv):
    in_maps = make_in_maps(student_feat, t_feat0, t_feat1, t_feat2,
                           Wq, bq, Wk, bk, Wv, bv)
    res = run(in_maps, trace=False)
    out = np.stack([unpack_out(res.results[b]["out"]) for b in range(B)])
    return out.astype(np.float32)


# revision 13
# speedup vs baseline: 1.0043x; 1.0043x over previous
"""CrossTeacherAttention Trainium2 kernel (fp8 DoubleRow rewrite).

Math per batch element b (x as [C=256, N=1024], N=H*W):
  Q  = Wq Xs + bq                       [C,N]
  G  = Wk^T Q                           [C,N]   (so S_t = Xt^T G: the three
                                                 K-projections fold into one)
  S_t[m,n] = sum_c Xt[c,m] G[c,n]
  E_t = exp(S_t/16 - 1.5)               (the -1.5 keeps E in fp8e4 range and
                                         cancels in the softmax normalization;
                                         bk shifts S per-n only -> provably no
                                         effect on the output, dropped)
  V_t^T = Xt^T Wv^T                     [N,C]  (bv deferred to the residual)
  O_t[n,c] (n-partition-major) = sum_m E_t[m,n] V_t^T[m,c], with a 257th
    moving column of constant 3.0 producing Z3_t[n] = 3*sum_m E_t[m,n] in the
    same PSUM accumulation.
  out^T = (Xs^T + bv) + sum_t O_t[:, :256] / Z3_t    (teacher weights are
    exactly 1/3: attn.mean(-1) of a softmax is 1/N, softmax over t of equal
    values is 1/3 -- folded into the 3.0 ones-column)

All five matmul families run as fp8e4 MatmulPerfMode.DoubleRow (K=256 per
instruction at 0.5 cycles/row).  Operands are packed [128, 2, F] with logical
contraction index k = p + 128*j.  exp runs on ACT as [128,1024] instructions
reading a 2-bank PSUM tile.  Normalization is per-partition (n on partitions):
DVE reciprocal of the fused Z3 column + scalar_tensor_tensor (O*recip + acc),
split across DVE and GPSIMD.  The residual Xs^T + bv is computed on host,
shipped as bf16 directly into the accumulator tile; output returns bf16
[128, 8, 256] (n-major) and the host unpacks/transposes/upcasts.

Sharding: data-parallel over batch, B=8 -> one batch element per core.
"""

import sys

sys.path.insert(0, "/opt/trn_rl_repo")

import ml_dtypes
import numpy as np

import concourse.bass as bass
import concourse.tile as tile
from concourse import mybir
from concourse.bass_utils import run_bass_kernel_spmd

B, C, H, W = 8, 256, 32, 32
N = H * W  # 1024
T = 3
P = 128
F32 = mybir.dt.float32
BF16 = mybir.dt.bfloat16
F8 = mybir.dt.float8e4
F8E5 = mybir.dt.float8e5
NP_F8 = ml_dtypes.float8_e4m3
NP_BF16 = ml_dtypes.bfloat16
SCALE = C ** -0.5  # 1/16
EXP_BIAS = -1.5
DR = mybir.MatmulPerfMode.DoubleRow


def build_nc():
    nc = bass.Bass()
    wpk_d = nc.dram_tensor("wpk", [P, 2, 2 * C], F8, kind="ExternalInput")
    xsp_d = nc.dram_tensor("xsp", [P, 2, N], F8, kind="ExternalInput")
    xt_d = nc.dram_tensor("xt", [T, P, 2, N], F8, kind="ExternalInput")
    gb_d = nc.dram_tensor("gb", [P, 2], F32, kind="ExternalInput")
    acc_d = nc.dram_tensor("accin", [P, 8, C], BF16, kind="ExternalInput")
    out_d = nc.dram_tensor("out", [P, 8, C], BF16, kind="ExternalOutput")

    with tile.TileContext(nc) as tc:
        with (
            tc.tile_pool(name="consts", bufs=1) as consts,
            tc.tile_pool(name="epool", bufs=12) as epool,
            tc.tile_pool(name="vpool", bufs=12) as vpool,
            tc.tile_pool(name="rpool", bufs=4) as rpool,
            tc.tile_pool(name="ps", bufs=2, space="PSUM") as ps,
            tc.tile_pool(name="pv", bufs=2, space="PSUM") as pv,
            tc.tile_pool(name="po", bufs=2, space="PSUM") as po,
        ):
            # ---- input loads (spread across engine DGE queues) ----
            wpk = consts.tile([P, 2, 2 * C], F8, tag="wpk", name="wpk")
            nc.sync.dma_start(out=wpk, in_=wpk_d[:, :, :])
            xsp = consts.tile([P, 2, N], F8, tag="xsp", name="xsp")
            nc.scalar.dma_start(out=xsp, in_=xsp_d[:, :, :])
            xt0 = consts.tile([P, 2, N], F8, tag="xt0", name="xt0")
            nc.gpsimd.dma_start(out=xt0, in_=xt_d[0])
            gb_sb = consts.tile([P, 2], F32, tag="gb", name="gb_sb")
            nc.sync.dma_start(out=gb_sb, in_=gb_d[:, :])
            acc = consts.tile([P, 8, C], BF16, tag="acc", name="acc")
            nc.scalar.dma_start(out=acc, in_=acc_d[:, :, :])
            xt1 = consts.tile([P, 2, N], F8, tag="xt1", name="xt1")
            nc.gpsimd.dma_start(out=xt1, in_=xt_d[1])
            xt2 = consts.tile([P, 2, N], F8, tag="xt2", name="xt2")
            nc.gpsimd.dma_start(out=xt2, in_=xt_d[2])
            xts = [xt0, xt1, xt2]

            ebias = consts.tile([P, 1], F32, tag="ebias", name="ebias")
            nc.vector.memset(ebias, EXP_BIAS)

            wm = wpk[:, :, 0:C]
            wv = wpk[:, :, C:2 * C]

            # ---- G = M Xs + gb (M = Wk^T Wq, gb = Wk^T bq, both host-side)
            # nh0 casts on DVE, nh1 on ACT so the two halves pipeline in
            # parallel during startup ----
            gf = consts.tile([P, 2, N], F8, tag="gf", name="gf")
            for nh in range(2):
                for co in range(2):
                    gp = ps.tile([P, 1024], F32, tag="s", name="gp")
                    nc.tensor.matmul(
                        gp[:, :512],
                        wm[:, :, co * P:(co + 1) * P],
                        xsp[:, :, nh * 512:(nh + 1) * 512],
                        start=True, stop=True, perf_mode=DR,
                    )
                    nc.vector.tensor_scalar_add(
                        gf[:, co, nh * 512:(nh + 1) * 512],
                        gp[:, :512],
                        gb_sb[:, co:co + 1],
                    )

            e_tiles = [[None] * 4 for _ in range(T)]
            v_tiles = [[None] * 4 for _ in range(T)]

            def emit_v(t):
                # V_t^T = Xt^T Wv^T -> v_aug [m-part, 2, 257] fp8 with a
                # 257th column of 3.0 (fused 3*Z row-sum weights)
                for r in range(4):
                    va = vpool.tile([P, 2, C + 1], F8, tag="v",
                                    name=f"v{t}{r}")
                    v_tiles[t][r] = va
                    nc.gpsimd.memset(va[:, :, C:C + 1], 3.0)
                    for j in range(2):
                        mi = 2 * r + j
                        vp = pv.tile([P, 512], F32, tag="v", name="vp")
                        nc.tensor.matmul(
                            vp[:, :C],
                            xts[t][:, :, mi * P:(mi + 1) * P],
                            wv,
                            start=True, stop=True, perf_mode=DR,
                        )
                        nc.vector.tensor_copy(va[:, j, :C], vp[:, :C])

            def emit_s(t):
                # S_t = Xt^T G -> exp -> packed e tiles [m-part, 2, 1024] fp8
                for mi in range(8):
                    r, j = divmod(mi, 2)
                    if j == 0:
                        e_tiles[t][r] = epool.tile([P, 2, N], F8E5, tag="e",
                                                   name=f"e{t}{r}")
                    sp = ps.tile([P, 1024], F32, tag="s", name="sp")
                    for nh in range(2):
                        nc.tensor.matmul(
                            sp[:, nh * 512:(nh + 1) * 512],
                            xts[t][:, :, mi * P:(mi + 1) * P],
                            gf[:, :, nh * 512:(nh + 1) * 512],
                            start=True, stop=True, perf_mode=DR,
                        )
                    if t == 0 and mi == 0:
                        # split halves: the first exp only waits on gf nh0
                        for nh in range(2):
                            nc.scalar.activation(
                                e_tiles[t][r][:, j, nh * 512:(nh + 1) * 512],
                                sp[:, nh * 512:(nh + 1) * 512],
                                func=mybir.ActivationFunctionType.Exp,
                                bias=ebias,
                                scale=SCALE,
                            )
                    else:
                        nc.scalar.activation(
                            e_tiles[t][r][:, j, :],
                            sp,
                            func=mybir.ActivationFunctionType.Exp,
                            bias=ebias,
                            scale=SCALE,
                        )

            def emit_o(t):
                # O_t [n-part, 257] += e_chunk^T @ v_aug over 4 m-pairs;
                # col 256 = 3*Z.  Normalize per-partition and accumulate.
                for w in range(2):
                    ops = []
                    for i in range(4):
                        opool, otag = (po, "o") if i % 2 == 0 else (pv, "v")
                        ops.append(opool.tile([P, 512], F32, tag=otag,
                                              name="op"))
                    for r in range(4):
                        for i in range(4):
                            nc.tensor.matmul(
                                ops[i][:, :C + 1],
                                e_tiles[t][r][:, :, (4 * w + i) * P:
                                              (4 * w + i + 1) * P],
                                v_tiles[t][r],
                                start=(r == 0), stop=(r == 3), perf_mode=DR,
                            )
                    for i in range(4):
                        nk = 4 * w + i
                        rp = rpool.tile([P, 1], F32, tag="r", name="rp")
                        nc.vector.reciprocal(rp, ops[i][:, C:C + 1])
                        nc.vector.scalar_tensor_tensor(
                            out=acc[:, nk, :],
                            in0=ops[i][:, :C],
                            scalar=rp,
                            in1=acc[:, nk, :],
                            op0=mybir.AluOpType.mult,
                            op1=mybir.AluOpType.add,
                        )

            # Emission order keeps PE streaming while ACT works through exps:
            # S(t0) V(t0) S(t1) V(t1) O(t0) S(t2) V(t2) O(t1) O(t2)
            emit_s(0)
            emit_v(0)
            emit_s(1)
            emit_v(1)
            emit_o(0)
            emit_s(2)
            emit_v(2)
            emit_o(1)
            emit_o(2)

            nc.sync.dma_start(out=out_d[:, 0:4, :], in_=acc[:, 0:4, :])
            nc.sync.dma_start(out=out_d[:, 4:6, :], in_=acc[:, 4:6, :])
            nc.sync.dma_start(out=out_d[:, 6:8, :], in_=acc[:, 6:8, :])

    _split_multi_waits(nc)
    if not nc.is_finalized():
        nc.finalize()
    return nc


def _split_multi_waits(nc):
    """walrus can encode at most one sync-wait per instruction. Hoist every
    wait of a multi-wait instruction onto single-wait nops on the same
    engine, placed immediately before it in program order."""
    fixes = []
    for fn in nc.m.functions:
        for blk in fn.blocks:
            for inst in blk.instructions:
                si = getattr(inst, "sync_info", None)
                if (si is not None and si.on_wait and len(si.on_wait) > 1
                        and getattr(inst, "engine", None) is not None):
                    fixes.append((blk, inst))
    for blk, inst in fixes:
        si = inst.sync_info
        waits = list(si.on_wait)
        nops = []
        for w in waits:
            nop = nc.engines[inst.engine].nop(nofuse=True).ins
            nop.sync_info = mybir.SyncInfo(on_wait=[w], on_update=[])
            nops.append(nop)
        inst.sync_info = mybir.SyncInfo(on_wait=[], on_update=list(si.on_update))
        nop_names = {n.name for n in nops}
        for fn2 in nc.m.functions:
            for blk2 in fn2.blocks:
                blk2.instructions = [
                    i for i in blk2.instructions if i.name not in nop_names
                ]
        pos = next(i for i, x in enumerate(blk.instructions)
                   if x.name == inst.name)
        blk.instructions = (blk.instructions[:pos] + nops
                            + blk.instructions[pos:])


_NC = None


def _get_nc():
    global _NC
    if _NC is None:
        _NC = build_nc()
    return _NC


def _pack2(a):
    """[256, X] row-major -> [128, 2, X] with row c at [c % 128, c // 128]."""
    return np.ascontiguousarray(a.reshape(2, P, -1).transpose(1, 0, 2))


def make_in_maps(student_feat, t_feat0, t_feat1, t_feat2,
                 Wq, bq, Wk, bk, Wv, bv):
    xs = np.asarray(student_feat, np.float32).reshape(B, C, N)
    xt = np.ascontiguousarray(
        np.stack([t_feat0, t_feat1, t_feat2], axis=1), np.float32
    ).reshape(B, T, C, N)
    wq32 = np.asarray(Wq, np.float32)
    wk32 = np.asarray(Wk, np.float32)
    m = wk32.T @ wq32  # G = M Xs + gb folds the Q projection away
    gb = wk32.T @ np.asarray(bq, np.float32)
    wpk = np.concatenate(
        [
            _pack2(m.T.astype(NP_F8)),
            _pack2(np.asarray(Wv, np.float32).T.astype(NP_F8)),
        ],
        axis=2,
    )
    gbp = np.ascontiguousarray(gb.reshape(2, P).T)
    bv32 = np.asarray(bv, np.float32)
    maps = []
    for b in range(B):
        xsp = _pack2(xs[b].astype(NP_F8))
        xtp = np.stack([_pack2(xt[b, t].astype(NP_F8)) for t in range(T)])
        accin = np.ascontiguousarray(
            (xs[b].T + bv32[None, :]).reshape(8, P, C).transpose(1, 0, 2)
        ).astype(NP_BF16)
        maps.append({"wpk": wpk, "xsp": xsp, "xt": xtp, "gb": gbp,
                     "accin": accin})
    return maps


def run(in_maps, trace=False):
    nc = _get_nc()
    return run_bass_kernel_spmd(nc, in_maps, core_ids=list(range(B)),
                                trace=trace)


def unpack_out(raw):
    """[128, 8, 256] bf16 n-major -> [C, H, W] f32."""
    o = np.asarray(raw).astype(np.float32).transpose(1, 0, 2).reshape(N, C)
    return np.ascontiguousarray(o.T).reshape(C, H, W)


def kernel(student_feat, t_feat0, t_feat1, t_feat2,
           Wq, bq, Wk, bk, Wv, bv):
    in_maps = make_in_maps(student_feat, t_feat0, t_feat1, t_feat2,
                           Wq, bq, Wk, bk, Wv, bv)
    res = run(in_maps, trace=False)
    out = np.stack([unpack_out(res.results[b]["out"]) for b in range(B)])
    return out.astype(np.float32)


# revision 14
# speedup vs baseline: 1.0161x; 1.0117x over previous
"""CrossTeacherAttention Trainium2 kernel (fp8 DoubleRow rewrite).

Math per batch element b (x as [C=256, N=1024], N=H*W):
  Q  = Wq Xs + bq                       [C,N]
  G  = Wk^T Q                           [C,N]   (so S_t = Xt^T G: the three
                                                 K-projections fold into one)
  S_t[m,n] = sum_c Xt[c,m] G[c,n]
  E_t = exp(S_t/16 - 1.5)               (the -1.5 keeps E in fp8e4 range and
                                         cancels in the softmax normalization;
                                         bk shifts S per-n only -> provably no
                                         effect on the output, dropped)
  V_t^T = Xt^T Wv^T                     [N,C]  (bv deferred to the residual)
  O_t[n,c] (n-partition-major) = sum_m E_t[m,n] V_t^T[m,c], with a 257th
    moving column of constant 3.0 producing Z3_t[n] = 3*sum_m E_t[m,n] in the
    same PSUM accumulation.
  out^T = (Xs^T + bv) + sum_t O_t[:, :256] / Z3_t    (teacher weights are
    exactly 1/3: attn.mean(-1) of a softmax is 1/N, softmax over t of equal
    values is 1/3 -- folded into the 3.0 ones-column)

All five matmul families run as fp8e4 MatmulPerfMode.DoubleRow (K=256 per
instruction at 0.5 cycles/row).  Operands are packed [128, 2, F] with logical
contraction index k = p + 128*j.  exp runs on ACT as [128,1024] instructions
reading a 2-bank PSUM tile.  Normalization is per-partition (n on partitions):
DVE reciprocal of the fused Z3 column + scalar_tensor_tensor (O*recip + acc),
split across DVE and GPSIMD.  The residual Xs^T + bv is computed on host,
shipped as bf16 directly into the accumulator tile; output returns bf16
[128, 8, 256] (n-major) and the host unpacks/transposes/upcasts.

Sharding: data-parallel over batch, B=8 -> one batch element per core.
"""

import sys

sys.path.insert(0, "/opt/trn_rl_repo")

import ml_dtypes
import numpy as np

import concourse.bass as bass
import concourse.tile as tile
from concourse import mybir
from concourse.bass_utils import run_bass_kernel_spmd

B, C, H, W = 8, 256, 32, 32
N = H * W  # 1024
T = 3
P = 128
F32 = mybir.dt.float32
BF16 = mybir.dt.bfloat16
F8 = mybir.dt.float8e4
F8E5 = mybir.dt.float8e5
NP_F8 = ml_dtypes.float8_e4m3
NP_BF16 = ml_dtypes.bfloat16
SCALE = C ** -0.5  # 1/16
EXP_BIAS = -1.5
DR = mybir.MatmulPerfMode.DoubleRow


def build_nc():
    nc = bass.Bass()
    wpk_d = nc.dram_tensor("wpk", [P, 2, 2 * C], F8, kind="ExternalInput")
    xsp_d = nc.dram_tensor("xsp", [P, 2, N], F8, kind="ExternalInput")
    xt_d = nc.dram_tensor("xt", [T, P, 2, N], F8, kind="ExternalInput")
    gb_d = nc.dram_tensor("gb", [P, 2], F32, kind="ExternalInput")
    acc_d = nc.dram_tensor("accin", [P, 8, C], BF16, kind="ExternalInput")
    out_d = nc.dram_tensor("out", [P, 8, C], BF16, kind="ExternalOutput")

    with tile.TileContext(nc) as tc:
        with (
            tc.tile_pool(name="consts", bufs=1) as consts,
            tc.tile_pool(name="epool", bufs=12) as epool,
            tc.tile_pool(name="vpool", bufs=12) as vpool,
            tc.tile_pool(name="rpool", bufs=4) as rpool,
            tc.tile_pool(name="ps", bufs=2, space="PSUM") as ps,
            tc.tile_pool(name="pv", bufs=2, space="PSUM") as pv,
            tc.tile_pool(name="po", bufs=2, space="PSUM") as po,
        ):
            # ---- input loads (spread across engine DGE queues) ----
            wpk = consts.tile([P, 2, 2 * C], F8, tag="wpk", name="wpk")
            nc.sync.dma_start(out=wpk, in_=wpk_d[:, :, :])
            xsp = consts.tile([P, 2, N], F8, tag="xsp", name="xsp")
            nc.scalar.dma_start(out=xsp, in_=xsp_d[:, :, :])
            xt0 = consts.tile([P, 2, N], F8, tag="xt0", name="xt0")
            nc.gpsimd.dma_start(out=xt0, in_=xt_d[0])
            gb_sb = consts.tile([P, 2], F32, tag="gb", name="gb_sb")
            nc.sync.dma_start(out=gb_sb, in_=gb_d[:, :])
            acc = consts.tile([P, 8, C], BF16, tag="acc", name="acc")
            nc.scalar.dma_start(out=acc, in_=acc_d[:, :, :])
            xt1 = consts.tile([P, 2, N], F8, tag="xt1", name="xt1")
            nc.gpsimd.dma_start(out=xt1, in_=xt_d[1])
            xt2 = consts.tile([P, 2, N], F8, tag="xt2", name="xt2")
            nc.gpsimd.dma_start(out=xt2, in_=xt_d[2])
            xts = [xt0, xt1, xt2]

            ebias = consts.tile([P, 1], F32, tag="ebias", name="ebias")
            nc.vector.memset(ebias, EXP_BIAS)

            wm = wpk[:, :, 0:C]
            wv = wpk[:, :, C:2 * C]

            # ---- G = M Xs + gb (M = Wk^T Wq, gb = Wk^T bq, both host-side)
            # nh0 casts on DVE, nh1 on ACT so the two halves pipeline in
            # parallel during startup ----
            gf = consts.tile([P, 2, N], F8, tag="gf", name="gf")
            for nh in range(2):
                for co in range(2):
                    gp = ps.tile([P, 1024], F32, tag="s", name="gp")
                    nc.tensor.matmul(
                        gp[:, :512],
                        wm[:, :, co * P:(co + 1) * P],
                        xsp[:, :, nh * 512:(nh + 1) * 512],
                        start=True, stop=True, perf_mode=DR,
                    )
                    if nh == 0:
                        nc.vector.tensor_scalar_add(
                            gf[:, co, nh * 512:(nh + 1) * 512],
                            gp[:, :512],
                            gb_sb[:, co:co + 1],
                        )
                    else:
                        nc.scalar.activation(
                            gf[:, co, nh * 512:(nh + 1) * 512],
                            gp[:, :512],
                            func=mybir.ActivationFunctionType.Identity,
                            bias=gb_sb[:, co:co + 1],
                        )

            e_tiles = [[None] * 4 for _ in range(T)]
            v_tiles = [[None] * 4 for _ in range(T)]

            def emit_v(t):
                # V_t^T = Xt^T Wv^T -> v_aug [m-part, 2, 257] fp8 with a
                # 257th column of 3.0 (fused 3*Z row-sum weights)
                for r in range(4):
                    va = vpool.tile([P, 2, C + 1], F8, tag="v",
                                    name=f"v{t}{r}")
                    v_tiles[t][r] = va
                    nc.gpsimd.memset(va[:, :, C:C + 1], 3.0)
                    for j in range(2):
                        mi = 2 * r + j
                        vp = pv.tile([P, 512], F32, tag="v", name="vp")
                        nc.tensor.matmul(
                            vp[:, :C],
                            xts[t][:, :, mi * P:(mi + 1) * P],
                            wv,
                            start=True, stop=True, perf_mode=DR,
                        )
                        nc.vector.tensor_copy(va[:, j, :C], vp[:, :C])

            def emit_s(t):
                # S_t = Xt^T G -> exp -> packed e tiles [m-part, 2, 1024] fp8
                for mi in range(8):
                    r, j = divmod(mi, 2)
                    if j == 0:
                        e_tiles[t][r] = epool.tile([P, 2, N], F8E5, tag="e",
                                                   name=f"e{t}{r}")
                    sp = ps.tile([P, 1024], F32, tag="s", name="sp")
                    for nh in range(2):
                        nc.tensor.matmul(
                            sp[:, nh * 512:(nh + 1) * 512],
                            xts[t][:, :, mi * P:(mi + 1) * P],
                            gf[:, :, nh * 512:(nh + 1) * 512],
                            start=True, stop=True, perf_mode=DR,
                        )
                    if t == 0 and mi == 0:
                        # split halves: the first exp only waits on gf nh0
                        for nh in range(2):
                            nc.scalar.activation(
                                e_tiles[t][r][:, j, nh * 512:(nh + 1) * 512],
                                sp[:, nh * 512:(nh + 1) * 512],
                                func=mybir.ActivationFunctionType.Exp,
                                bias=ebias,
                                scale=SCALE,
                            )
                    else:
                        nc.scalar.activation(
                            e_tiles[t][r][:, j, :],
                            sp,
                            func=mybir.ActivationFunctionType.Exp,
                            bias=ebias,
                            scale=SCALE,
                        )

            def emit_o(t):
                # O_t [n-part, 257] += e_chunk^T @ v_aug over 4 m-pairs;
                # col 256 = 3*Z.  Normalize per-partition and accumulate.
                for w in range(2):
                    ops = []
                    for i in range(4):
                        opool, otag = (po, "o") if i % 2 == 0 else (pv, "v")
                        ops.append(opool.tile([P, 512], F32, tag=otag,
                                              name="op"))
                    for r in range(4):
                        for i in range(4):
                            nc.tensor.matmul(
                                ops[i][:, :C + 1],
                                e_tiles[t][r][:, :, (4 * w + i) * P:
                                              (4 * w + i + 1) * P],
                                v_tiles[t][r],
                                start=(r == 0), stop=(r == 3), perf_mode=DR,
                            )
                    for i in range(4):
                        nk = 4 * w + i
                        rp = rpool.tile([P, 1], F32, tag="r", name="rp")
                        nc.vector.reciprocal(rp, ops[i][:, C:C + 1])
                        nc.vector.scalar_tensor_tensor(
                            out=acc[:, nk, :],
                            in0=ops[i][:, :C],
                            scalar=rp,
                            in1=acc[:, nk, :],
                            op0=mybir.AluOpType.mult,
                            op1=mybir.AluOpType.add,
                        )

            # Emission order keeps PE streaming while ACT works through exps:
            # S(t0) V(t0) S(t1) V(t1) O(t0) S(t2) V(t2) O(t1) O(t2)
            emit_s(0)
            emit_v(0)
            emit_s(1)
            emit_v(1)
            emit_o(0)
            emit_s(2)
            emit_v(2)
            emit_o(1)
            emit_o(2)

            nc.sync.dma_start(out=out_d[:, 0:4, :], in_=acc[:, 0:4, :])
            nc.sync.dma_start(out=out_d[:, 4:6, :], in_=acc[:, 4:6, :])
            nc.sync.dma_start(out=out_d[:, 6:8, :], in_=acc[:, 6:8, :])

    _split_multi_waits(nc)
    if not nc.is_finalized():
        nc.finalize()
    return nc


def _split_multi_waits(nc):
    """walrus can encode at most one sync-wait per instruction. Hoist every
    wait of a multi-wait instruction onto single-wait nops on the same
    engine, placed immediately before it in program order."""
    fixes = []
    for fn in nc.m.functions:
        for blk in fn.blocks:
            for inst in blk.instructions:
                si = getattr(inst, "sync_info", None)
                if (si is not None and si.on_wait and len(si.on_wait) > 1
                        and getattr(inst, "engine", None) is not None):
                    fixes.append((blk, inst))
    for blk, inst in fixes:
        si = inst.sync_info
        waits = list(si.on_wait)
        nops = []
        for w in waits:
            nop = nc.engines[inst.engine].nop(nofuse=True).ins
            nop.sync_info = mybir.SyncInfo(on_wait=[w], on_update=[])
            nops.append(nop)
        inst.sync_info = mybir.SyncInfo(on_wait=[], on_update=list(si.on_update))
        nop_names = {n.name for n in nops}
        for fn2 in nc.m.functions:
            for blk2 in fn2.blocks:
                blk2.instructions = [
                    i for i in blk2.instructions if i.name not in nop_names
                ]
        pos = next(i for i, x in enumerate(blk.instructions)
                   if x.name == inst.name)
        blk.instructions = (blk.instructions[:pos] + nops
                            + blk.instructions[pos:])


_NC = None


def _get_nc():
    global _NC
    if _NC is None:
        _NC = build_nc()
    return _NC


def _pack2(a):
    """[256, X] row-major -> [128, 2, X] with row c at [c % 128, c // 128]."""
    return np.ascontiguousarray(a.reshape(2, P, -1).transpose(1, 0, 2))


def make_in_maps(student_feat, t_feat0, t_feat1, t_feat2,
                 Wq, bq, Wk, bk, Wv, bv):
    xs = np.asarray(student_feat, np.float32).reshape(B, C, N)
    xt = np.ascontiguousarray(
        np.stack([t_feat0, t_feat1, t_feat2], axis=1), np.float32
    ).reshape(B, T, C, N)
    wq32 = np.asarray(Wq, np.float32)
    wk32 = np.asarray(Wk, np.float32)
    m = wk32.T @ wq32  # G = M Xs + gb folds the Q projection away
    gb = wk32.T @ np.asarray(bq, np.float32)
    wpk = np.concatenate(
        [
            _pack2(m.T.astype(NP_F8)),
            _pack2(np.asarray(Wv, np.float32).T.astype(NP_F8)),
        ],
        axis=2,
    )
    gbp = np.ascontiguousarray(gb.reshape(2, P).T)
    bv32 = np.asarray(bv, np.float32)
    maps = []
    for b in range(B):
        xsp = _pack2(xs[b].astype(NP_F8))
        xtp = np.stack([_pack2(xt[b, t].astype(NP_F8)) for t in range(T)])
        accin = np.ascontiguousarray(
            (xs[b].T + bv32[None, :]).reshape(8, P, C).transpose(1, 0, 2)
        ).astype(NP_BF16)
        maps.append({"wpk": wpk, "xsp": xsp, "xt": xtp, "gb": gbp,
                     "accin": accin})
    return maps


def run(in_maps, trace=False):
    nc = _get_nc()
    return run_bass_kernel_spmd(nc, in_maps, core_ids=list(range(B)),
                                trace=trace)


def unpack_out(raw):
    """[128, 8, 256] bf16 n-major -> [C, H, W] f32."""
    o = np.asarray(raw).astype(np.float32).transpose(1, 0, 2).reshape(N, C)
    return np.ascontiguousarray(o.T).reshape(C, H, W)


def kernel(student_feat, t_feat0, t_feat1, t_feat2,
           Wq, bq, Wk, bk, Wv, bv):
    in_maps = make_in_maps(student_feat, t_feat0, t_feat1, t_feat2,
                           Wq, bq, Wk, bk, Wv, bv)
    res = run(in_maps, trace=False)
    out = np.stack([unpack_out(res.results[b]["out"]) for b in range(B)])
    return out.astype(np.float32)


# revision 15
# speedup vs baseline: 1.0211x; 1.0049x over previous
"""CrossTeacherAttention Trainium2 kernel (fp8 DoubleRow rewrite).

Math per batch element b (x as [C=256, N=1024], N=H*W):
  Q  = Wq Xs + bq                       [C,N]
  G  = Wk^T Q                           [C,N]   (so S_t = Xt^T G: the three
                                                 K-projections fold into one)
  S_t[m,n] = sum_c Xt[c,m] G[c,n]
  E_t = exp(S_t/16 - 1.5)               (the -1.5 keeps E in fp8e4 range and
                                         cancels in the softmax normalization;
                                         bk shifts S per-n only -> provably no
                                         effect on the output, dropped)
  V_t^T = Xt^T Wv^T                     [N,C]  (bv deferred to the residual)
  O_t[n,c] (n-partition-major) = sum_m E_t[m,n] V_t^T[m,c], with a 257th
    moving column of constant 3.0 producing Z3_t[n] = 3*sum_m E_t[m,n] in the
    same PSUM accumulation.
  out^T = (Xs^T + bv) + sum_t O_t[:, :256] / Z3_t    (teacher weights are
    exactly 1/3: attn.mean(-1) of a softmax is 1/N, softmax over t of equal
    values is 1/3 -- folded into the 3.0 ones-column)

All five matmul families run as fp8e4 MatmulPerfMode.DoubleRow (K=256 per
instruction at 0.5 cycles/row).  Operands are packed [128, 2, F] with logical
contraction index k = p + 128*j.  exp runs on ACT as [128,1024] instructions
reading a 2-bank PSUM tile.  Normalization is per-partition (n on partitions):
DVE reciprocal of the fused Z3 column + scalar_tensor_tensor (O*recip + acc),
split across DVE and GPSIMD.  The residual Xs^T + bv is computed on host,
shipped as bf16 directly into the accumulator tile; output returns bf16
[128, 8, 256] (n-major) and the host unpacks/transposes/upcasts.

Sharding: data-parallel over batch, B=8 -> one batch element per core.
"""

import sys

sys.path.insert(0, "/opt/trn_rl_repo")

import ml_dtypes
import numpy as np

import concourse.bass as bass
import concourse.tile as tile
from concourse import mybir
from concourse.bass_utils import run_bass_kernel_spmd

B, C, H, W = 8, 256, 32, 32
N = H * W  # 1024
T = 3
P = 128
F32 = mybir.dt.float32
BF16 = mybir.dt.bfloat16
F8 = mybir.dt.float8e4
F8E5 = mybir.dt.float8e5
NP_F8 = ml_dtypes.float8_e4m3
NP_BF16 = ml_dtypes.bfloat16
SCALE = C ** -0.5  # 1/16
EXP_BIAS = -1.5
DR = mybir.MatmulPerfMode.DoubleRow


def build_nc():
    nc = bass.Bass()
    wpk_d = nc.dram_tensor("wpk", [P, 2, 2 * C], F8, kind="ExternalInput")
    xsp_d = nc.dram_tensor("xsp", [P, 2, N], F8, kind="ExternalInput")
    xt_d = nc.dram_tensor("xt", [T, P, 2, N], F8, kind="ExternalInput")
    gb_d = nc.dram_tensor("gb", [P, 2], F32, kind="ExternalInput")
    acc_d = nc.dram_tensor("accin", [P, 8, C], BF16, kind="ExternalInput")
    out_d = nc.dram_tensor("out", [P, 8, C], BF16, kind="ExternalOutput")

    with tile.TileContext(nc) as tc:
        with (
            tc.tile_pool(name="consts", bufs=1) as consts,
            tc.tile_pool(name="epool", bufs=12) as epool,
            tc.tile_pool(name="vpool", bufs=12) as vpool,
            tc.tile_pool(name="rpool", bufs=4) as rpool,
            tc.tile_pool(name="ps", bufs=2, space="PSUM") as ps,
            tc.tile_pool(name="pv", bufs=2, space="PSUM") as pv,
            tc.tile_pool(name="po", bufs=2, space="PSUM") as po,
        ):
            # ---- input loads (spread across engine DGE queues) ----
            wpk = consts.tile([P, 2, 2 * C], F8, tag="wpk", name="wpk")
            nc.sync.dma_start(out=wpk, in_=wpk_d[:, :, :])
            xsp = consts.tile([P, 2, N], F8, tag="xsp", name="xsp")
            nc.scalar.dma_start(out=xsp, in_=xsp_d[:, :, :])
            xt0 = consts.tile([P, 2, N], F8, tag="xt0", name="xt0")
            nc.gpsimd.dma_start(out=xt0, in_=xt_d[0])
            gb_sb = consts.tile([P, 2], F32, tag="gb", name="gb_sb")
            nc.sync.dma_start(out=gb_sb, in_=gb_d[:, :])
            acc = consts.tile([P, 8, C], BF16, tag="acc", name="acc")
            nc.scalar.dma_start(out=acc, in_=acc_d[:, :, :])
            xt1 = consts.tile([P, 2, N], F8, tag="xt1", name="xt1")
            nc.gpsimd.dma_start(out=xt1, in_=xt_d[1])
            xt2 = consts.tile([P, 2, N], F8, tag="xt2", name="xt2")
            nc.gpsimd.dma_start(out=xt2, in_=xt_d[2])
            xts = [xt0, xt1, xt2]

            ebias = consts.tile([P, 1], F32, tag="ebias", name="ebias")
            nc.vector.memset(ebias, EXP_BIAS)

            wm = wpk[:, :, 0:C]
            wv = wpk[:, :, C:2 * C]

            # ---- G = M Xs + gb (M = Wk^T Wq, gb = Wk^T bq, both host-side)
            # nh0 casts on DVE, nh1 on ACT so the two halves pipeline in
            # parallel during startup ----
            gf = consts.tile([P, 2, N], F8, tag="gf", name="gf")
            for nh in range(2):
                for co in range(2):
                    gp = ps.tile([P, 1024], F32, tag="s", name="gp")
                    nc.tensor.matmul(
                        gp[:, :512],
                        wm[:, :, co * P:(co + 1) * P],
                        xsp[:, :, nh * 512:(nh + 1) * 512],
                        start=True, stop=True, perf_mode=DR,
                    )
                    if nh == 0:
                        nc.vector.tensor_scalar_add(
                            gf[:, co, nh * 512:(nh + 1) * 512],
                            gp[:, :512],
                            gb_sb[:, co:co + 1],
                        )
                    else:
                        nc.scalar.activation(
                            gf[:, co, nh * 512:(nh + 1) * 512],
                            gp[:, :512],
                            func=mybir.ActivationFunctionType.Identity,
                            bias=gb_sb[:, co:co + 1],
                        )

            e_tiles = [[None] * 4 for _ in range(T)]
            v_tiles = [[None] * 4 for _ in range(T)]

            def emit_v(t):
                # V_t^T = Xt^T Wv^T -> v_aug [m-part, 2, 257] fp8 with a
                # 257th column of 3.0 (fused 3*Z row-sum weights)
                for r in range(4):
                    va = vpool.tile([P, 2, C + 1], F8, tag="v",
                                    name=f"v{t}{r}")
                    v_tiles[t][r] = va
                    nc.gpsimd.memset(va[:, :, C:C + 1], 3.0)
                    for j in range(2):
                        mi = 2 * r + j
                        vp = pv.tile([P, 512], F32, tag="v", name="vp")
                        nc.tensor.matmul(
                            vp[:, :C],
                            xts[t][:, :, mi * P:(mi + 1) * P],
                            wv,
                            start=True, stop=True, perf_mode=DR,
                        )
                        nc.vector.tensor_copy(va[:, j, :C], vp[:, :C])

            def emit_s(t):
                # S_t = Xt^T G -> exp -> packed e tiles [m-part, 2, 1024] fp8
                for mi in range(8):
                    r, j = divmod(mi, 2)
                    if j == 0:
                        e_tiles[t][r] = epool.tile([P, 2, N], F8E5, tag="e",
                                                   name=f"e{t}{r}")
                    sp = ps.tile([P, 1024], F32, tag="s", name="sp")
                    for nh in range(2):
                        nc.tensor.matmul(
                            sp[:, nh * 512:(nh + 1) * 512],
                            xts[t][:, :, mi * P:(mi + 1) * P],
                            gf[:, :, nh * 512:(nh + 1) * 512],
                            start=True, stop=True, perf_mode=DR,
                        )
                    nc.scalar.activation(
                        e_tiles[t][r][:, j, :],
                        sp,
                        func=mybir.ActivationFunctionType.Exp,
                        bias=ebias,
                        scale=SCALE,
                    )

            def emit_o(t):
                # O_t [n-part, 257] += e_chunk^T @ v_aug over 4 m-pairs;
                # col 256 = 3*Z.  Normalize per-partition and accumulate.
                for w in range(2):
                    ops = []
                    for i in range(4):
                        opool, otag = (po, "o") if i % 2 == 0 else (pv, "v")
                        ops.append(opool.tile([P, 512], F32, tag=otag,
                                              name="op"))
                    for r in range(4):
                        for i in range(4):
                            nc.tensor.matmul(
                                ops[i][:, :C + 1],
                                e_tiles[t][r][:, :, (4 * w + i) * P:
                                              (4 * w + i + 1) * P],
                                v_tiles[t][r],
                                start=(r == 0), stop=(r == 3), perf_mode=DR,
                            )
                    for i in range(4):
                        nk = 4 * w + i
                        rp = rpool.tile([P, 1], F32, tag="r", name="rp")
                        nc.vector.reciprocal(rp, ops[i][:, C:C + 1])
                        nc.vector.scalar_tensor_tensor(
                            out=acc[:, nk, :],
                            in0=ops[i][:, :C],
                            scalar=rp,
                            in1=acc[:, nk, :],
                            op0=mybir.AluOpType.mult,
                            op1=mybir.AluOpType.add,
                        )

            # Emission order keeps PE streaming while ACT works through exps:
            # S(t0) V(t0) S(t1) V(t1) O(t0) S(t2) V(t2) O(t1) O(t2)
            emit_s(0)
            emit_v(0)
            emit_s(1)
            emit_v(1)
            emit_o(0)
            emit_s(2)
            emit_v(2)
            emit_o(1)
            emit_o(2)

            nc.sync.dma_start(out=out_d[:, 0:4, :], in_=acc[:, 0:4, :])
            nc.sync.dma_start(out=out_d[:, 4:6, :], in_=acc[:, 4:6, :])
            nc.sync.dma_start(out=out_d[:, 6:8, :], in_=acc[:, 6:8, :])

    _split_multi_waits(nc)
    if not nc.is_finalized():
        nc.finalize()
    return nc


def _split_multi_waits(nc):
    """walrus can encode at most one sync-wait per instruction. Hoist every
    wait of a multi-wait instruction onto single-wait nops on the same
    engine, placed immediately before it in program order."""
    fixes = []
    for fn in nc.m.functions:
        for blk in fn.blocks:
            for inst in blk.instructions:
                si = getattr(inst, "sync_info", None)
                if (si is not None and si.on_wait and len(si.on_wait) > 1
                        and getattr(inst, "engine", None) is not None):
                    fixes.append((blk, inst))
    for blk, inst in fixes:
        si = inst.sync_info
        waits = list(si.on_wait)
        nops = []
        for w in waits:
            nop = nc.engines[inst.engine].nop(nofuse=True).ins
            nop.sync_info = mybir.SyncInfo(on_wait=[w], on_update=[])
            nops.append(nop)
        inst.sync_info = mybir.SyncInfo(on_wait=[], on_update=list(si.on_update))
        nop_names = {n.name for n in nops}
        for fn2 in nc.m.functions:
            for blk2 in fn2.blocks:
                blk2.instructions = [
                    i for i in blk2.instructions if i.name not in nop_names
                ]
        pos = next(i for i, x in enumerate(blk.instructions)
                   if x.name == inst.name)
        blk.instructions = (blk.instructions[:pos] + nops
                            + blk.instructions[pos:])


_NC = None


def _get_nc():
    global _NC
    if _NC is None:
        _NC = build_nc()
    return _NC


def _pack2(a):
    """[256, X] row-major -> [128, 2, X] with row c at [c % 128, c // 128]."""
    return np.ascontiguousarray(a.reshape(2, P, -1).transpose(1, 0, 2))


def make_in_maps(student_feat, t_feat0, t_feat1, t_feat2,
                 Wq, bq, Wk, bk, Wv, bv):
    xs = np.asarray(student_feat, np.float32).reshape(B, C, N)
    xt = np.ascontiguousarray(
        np.stack([t_feat0, t_feat1, t_feat2], axis=1), np.float32
    ).reshape(B, T, C, N)
    wq32 = np.asarray(Wq, np.float32)
    wk32 = np.asarray(Wk, np.float32)
    m = wk32.T @ wq32  # G = M Xs + gb folds the Q projection away
    gb = wk32.T @ np.asarray(bq, np.float32)
    wpk = np.concatenate(
        [
            _pack2(m.T.astype(NP_F8)),
            _pack2(np.asarray(Wv, np.float32).T.astype(NP_F8)),
        ],
        axis=2,
    )
    gbp = np.ascontiguousarray(gb.reshape(2, P).T)
    bv32 = np.asarray(bv, np.float32)
    maps = []
    for b in range(B):
        xsp = _pack2(xs[b].astype(NP_F8))
        xtp = np.stack([_pack2(xt[b, t].astype(NP_F8)) for t in range(T)])
        accin = np.ascontiguousarray(
            (xs[b].T + bv32[None, :]).reshape(8, P, C).transpose(1, 0, 2)
        ).astype(NP_BF16)
        maps.append({"wpk": wpk, "xsp": xsp, "xt": xtp, "gb": gbp,
                     "accin": accin})
    return maps


def run(in_maps, trace=False):
    nc = _get_nc()
    return run_bass_kernel_spmd(nc, in_maps, core_ids=list(range(B)),
                                trace=trace)


def unpack_out(raw):
    """[128, 8, 256] bf16 n-major -> [C, H, W] f32."""
    o = np.asarray(raw).astype(np.float32).transpose(1, 0, 2).reshape(N, C)
    return np.ascontiguousarray(o.T).reshape(C, H, W)


def kernel(student_feat, t_feat0, t_feat1, t_feat2,
           Wq, bq, Wk, bk, Wv, bv):
    in_maps = make_in_maps(student_feat, t_feat0, t_feat1, t_feat2,
                           Wq, bq, Wk, bk, Wv, bv)
    res = run(in_maps, trace=False)
    out = np.stack([unpack_out(res.results[b]["out"]) for b in range(B)])
    return out.astype(np.float32)
